# revision 13
# baseline (speedup 1.0000x reference)
"""Trainium2 Bass kernel for DAAttention (PAM + CAM dual attention, L=3 layer
pairs, B=8, C=64, N=1024), batch-parallel across 8 NeuronCores.

Per core (one batch element):
  - 6 projection convs (stride-2 3x3 for layer 0 as 9 tap matmuls over a
    host-padded input; 1x1 convs as chunked matmuls) -> S[i], T[i] [64,1024]
  - PAM: eT[i,j] = k[j]^T q[i] computed transposed (m on partitions) so both
    the softmax normalizer and the attention-value product are matmuls.
    exp without max subtraction (logits ~N(0,8), fp32-safe); row sums ride
    the value matmul through an appended ones row; normalization applied as
    a per-j rescale of the value-product output.
  - CAM: e_cam[i,j] = T[i] S[j]^T via PE-transposed copies; standard
    free-axis softmax; attention applied via one more matmul.
  - alpha ([3,3] softmaxed means) is reconstructed on the host from the
    exact S/T outputs (means commute with the bilinear forms).
"""

import os
import sys

import numpy as np

for _p in ("/opt/trn_rl_repo", "/root/.axon_site/_ro/trn_rl_repo"):
    if os.path.isdir(_p) and _p not in sys.path:
        sys.path.insert(0, _p)

import ml_dtypes
import concourse.bacc as bacc
import concourse.bass as bass  # noqa: F401
import concourse.mybir as mybir
from concourse import tile
from concourse.bass_utils import run_bass_kernel_spmd

L, C, H, W = 3, 64, 32, 32
N = H * W  # 1024
B = 8
F32 = mybir.dt.float32
R = mybir.dt.float32r
BF = mybir.dt.bfloat16
AF = mybir.ActivationFunctionType
ALU = mybir.AluOpType
AX = mybir.AxisListType

_CACHED = {}


def build_bass():
    nc = bacc.Bacc("TRN2", target_bir_lowering=False, debug=False, num_devices=B)

    # ---- DRAM parameters (per-core inputs) ----
    s0p = nc.dram_tensor("s0p", [128, 66, 66], R, kind="ExternalInput")
    t0p = nc.dram_tensor("t0p", [128, 66, 66], R, kind="ExternalInput")
    s1 = nc.dram_tensor("s1", [2, 128, 1024], R, kind="ExternalInput")
    t1 = nc.dram_tensor("t1", [2, 128, 1024], R, kind="ExternalInput")
    s2 = nc.dram_tensor("s2", [4, 128, 1024], R, kind="ExternalInput")
    t2 = nc.dram_tensor("t2", [4, 128, 1024], R, kind="ExternalInput")
    # packed constants: one DMA each
    # wall: [128, 30, 64] = w0s(9) w0t(9) w1s(2) w1t(2) w2s(4) w2t(4)
    wall = nc.dram_tensor("wall", [128, 30, 64], R, kind="ExternalInput")
    # pall: [65, 774] = pq(3x64) pk(3x64) cv(3x64) pv(3x66)
    pall = nc.dram_tensor("pall", [65, 774], R, kind="ExternalInput")
    # cnst: [64, 70] = cbias(6) identity(64)
    cnst = nc.dram_tensor("cnst", [64, 70], R, kind="ExternalInput")
    onesr = nc.dram_tensor("onesr", [1, 1024], R, kind="ExternalInput")
    identb = nc.dram_tensor("identb", [64, 64], BF, kind="ExternalInput")

    srcs = nc.dram_tensor("srcs", [L, C * N], F32, kind="ExternalOutput")
    tgts = nc.dram_tensor("tgts", [2 * L, C * N], F32, kind="ExternalOutput")
    srcs_r = srcs.ap().rearrange("l (c n) -> l c n", c=C)
    tgts_r = tgts.ap().rearrange("l (c n) -> l c n", c=C)

    with tile.TileContext(nc) as tc:
        with (
            tc.tile_pool(name="persist", bufs=1) as pers,
            tc.tile_pool(name="wpool", bufs=1) as wpool,
            tc.tile_pool(name="big", bufs=3) as bigp,
            tc.tile_pool(name="work", bufs=2) as work,
        ):
            # ---- persistent SBUF tensors ----
            Saug = [pers.tile([65, N], R, tag=f"Saug{i}", name=f"Saug{i}") for i in range(L)]
            Taug = [pers.tile([65, N], R, tag=f"Taug{i}", name=f"Taug{i}") for i in range(L)]
            q_sb = [pers.tile([128, N], R, tag=f"q{i}", name=f"q{i}") for i in range(L)]
            k_sb = [pers.tile([128, N], R, tag=f"k{i}", name=f"k{i}") for i in range(L)]
            vc_sb = [pers.tile([64, N], BF, tag=f"vc{i}", name=f"vc{i}") for i in range(L)]
            vT_sb = [pers.tile([128, 8, 66], BF, tag=f"vT{i}", name=f"vT{i}") for i in range(L)]
            TTb = [pers.tile([128, 8, 64], BF, tag=f"TT{i}", name=f"TT{i}") for i in range(L)]
            STb = [pers.tile([128, 8, 64], BF, tag=f"ST{i}", name=f"ST{i}") for i in range(L)]
            camv = [pers.tile([64, N], F32, tag=f"camv{i}", name=f"camv{i}") for i in range(L)]
            acc = [pers.tile([64, N], F32, tag=f"acc{i}", name=f"acc{i}") for i in range(L)]

            # ---- input loads first (conv inputs are the critical path);
            # constants in a handful of packed DMAs ----
            big_xin = {}
            for nm, hndl, sz in (("s0", s0p, 66 * 66), ("t0", t0p, 66 * 66),
                                 ("s1", s1, 2048), ("t1", t1, 2048),
                                 ("s2", s2, 4096), ("t2", t2, 4096)):
                xin = bigp.tile([128, sz], R, tag="big", name=f"xin_{nm}")
                big_xin[nm] = xin
                # split across partition quarters -> parallel DMA queues
                for p0 in range(0, 128, 32):
                    if sz == 66 * 66:
                        nc.sync.dma_start(
                            xin[p0:p0 + 32, :],
                            hndl.ap().rearrange("p a b -> p (a b)")[p0:p0 + 32])
                    else:
                        nk = sz // 1024
                        nc.sync.dma_start(
                            xin[p0:p0 + 32, :].rearrange(
                                "p (k n) -> p k n", k=nk),
                            hndl.ap().transpose([1, 0, 2])[p0:p0 + 32])

            wall_sb = wpool.tile([128, 30, 64], R, tag="wall")
            nc.sync.dma_start(wall_sb[:], wall.ap())
            pall_sb = wpool.tile([65, 774], R, tag="pall")
            nc.sync.dma_start(pall_sb[:], pall.ap())
            cnst_sb = wpool.tile([64, 70], R, tag="cnst")
            nc.sync.dma_start(cnst_sb[:], cnst.ap())
            idb = wpool.tile([64, 64], BF, tag="idb")
            nc.sync.dma_start(idb[:], identb.ap())

            cb = cnst_sb[:, 0:6].bitcast(F32)
            idr = cnst_sb[:, 6:70]
            paug = {}
            for gi, nm in enumerate(("pq", "pk", "cv")):
                for i in range(L):
                    paug[nm, i] = pall_sb[:, (gi * 3 + i) * 64:
                                          (gi * 3 + i + 1) * 64]
            for i in range(L):
                paug["pv", i] = pall_sb[:, 576 + i * 66:576 + (i + 1) * 66]
            woff = {"w0s": 0, "w0t": 9, "w1s": 18, "w1t": 20,
                    "w2s": 22, "w2t": 26}
            wsb = {nm: wall_sb[:, o:o + {"w0s": 9, "w0t": 9, "w1s": 2,
                                         "w1t": 2, "w2s": 4, "w2t": 4}[nm], :]
                   for nm, o in woff.items()}

            with tc.tile_pool(name="ps_big", bufs=2, space="PSUM") as ps_big, \
                 tc.tile_pool(name="ps_small", bufs=4, space="PSUM") as ps_small:

                # ================= Phase 1: convs =================
                def conv0(dst, xin, wname, bcol):
                    xv = xin.rearrange("p (a b) -> p a b", a=66)
                    pt = ps_big.tile([64, N], F32, tag="pbig")
                    pt3 = pt[:].rearrange("c (h w) -> c h w", h=32)
                    for ti, (kh, kw) in enumerate(
                            (a, b) for a in range(3) for b in range(3)):
                        for h0 in (0, 16):
                            nc.tensor.matmul(
                                pt3[:, h0:h0 + 16, :],
                                wsb[wname][:, kh * 3 + kw, :],
                                xv[:, kh + 2 * h0:kh + 2 * h0 + 32:2,
                                   kw:kw + 64:2],
                                start=(ti == 0), stop=(ti == 8),
                            )
                    nc.scalar.activation(dst[0:64, :], pt[:], AF.Identity,
                                         bias=cb[:, bcol:bcol + 1])
                    nc.sync.dma_start(dst[64:65, :], onesr.ap())

                def conv1x1(dst, xin, wname, nk, bcol):
                    xv = xin.rearrange("p (k n) -> p k n", k=nk)
                    pt = ps_big.tile([64, N], F32, tag="pbig")
                    for kc in range(nk):
                        for h in (0, 512):
                            nc.tensor.matmul(
                                pt[:, h:h + 512],
                                wsb[wname][:, kc, :],
                                xv[:, kc, h:h + 512],
                                start=(kc == 0), stop=(kc == nk - 1),
                            )
                    nc.scalar.activation(dst[0:64, :], pt[:], AF.Identity,
                                         bias=cb[:, bcol:bcol + 1])
                    nc.sync.dma_start(dst[64:65, :], onesr.ap())

                conv0(Saug[0], big_xin["s0"][:], "w0s", 0)
                conv0(Taug[0], big_xin["t0"][:], "w0t", 1)
                conv1x1(Saug[1], big_xin["s1"][:], "w1s", 2, 2)
                conv1x1(Taug[1], big_xin["t1"][:], "w1t", 2, 3)
                conv1x1(Saug[2], big_xin["s2"][:], "w2s", 4, 4)
                conv1x1(Taug[2], big_xin["t2"][:], "w2t", 4, 5)

                # write out S (srcs) and T (odd rows of tgts)
                for i in range(L):
                    nc.sync.dma_start(srcs_r[i], Saug[i][0:64, :].bitcast(F32))
                    nc.sync.dma_start(tgts_r[2 * i + 1],
                                      Taug[i][0:64, :].bitcast(F32))

                # ================= Phase 1b: projections =================
                def proj(dst, aug, xaug, replicate=False):
                    pt = ps_big.tile([64, N], F32, tag="pbig")
                    for h in (0, 512):
                        nc.tensor.matmul(pt[:, h:h + 512], aug,
                                         xaug[:, h:h + 512])
                    nc.vector.tensor_copy(dst[0:64, :], pt[:])
                    if replicate:
                        nc.sync.dma_start(dst[64:128, :], dst[0:64, :])

                for i in range(L):
                    proj(q_sb[i][:], paug["pq", i][:], Taug[i][:],
                         replicate=True)
                    proj(k_sb[i][:], paug["pk", i][:], Saug[i][:],
                         replicate=True)
                    proj(vc_sb[i][:], paug["cv", i][:], Taug[i][:])

                # transposed S/T (bf16) for CAM
                for i in range(L):
                    for (dst, srcT) in ((TTb[i], Taug[i]), (STb[i], Saug[i])):
                        pt = ps_small.tile([128, 512], R, tag="psmall")
                        for mc in range(8):
                            nc.tensor.transpose(
                                pt[:, mc * 64:(mc + 1) * 64],
                                srcT[0:64, mc * 128:(mc + 1) * 128],
                                idr[:],
                            )
                        nc.vector.tensor_copy(dst[:], pt[:])

                # ================= Phase 1c: CAM energies + softmax =========
                # (PE emits all 9 energy matmul groups back-to-back; the
                # DVE/ACT softmax chains trail behind while PE moves on to
                # the vT matmuls below.)
                attns = []
                for i in range(L):
                    attn = work.tile([64, 64], F32, tag=f"cam_attn{i}",
                                     name=f"cam_attn{i}", bufs=1)
                    attns.append(attn)
                    for j in range(L):
                        pe = ps_small.tile([64, 64], F32, tag="psmall")
                        for nt in range(8):
                            nc.tensor.matmul(pe[:], TTb[i][:, nt, :],
                                             STb[j][:, nt, :],
                                             start=(nt == 0), stop=(nt == 7))
                        negmax = work.tile([64, 1], F32, tag="cam_negmax")
                        nc.vector.tensor_reduce(negmax[:], pe[:], axis=AX.X,
                                                op=ALU.max, negate=True)
                        esb = work.tile([64, 64], F32, tag="cam_exp")
                        ssum = work.tile([64, 1], F32, tag="cam_sum")
                        nc.scalar.activation(esb[:], pe[:], AF.Exp,
                                             bias=negmax[:], accum_out=ssum[:])
                        inv = work.tile([64, 1], F32, tag="cam_inv")
                        nc.vector.reciprocal(inv[:], ssum[:])
                        nc.vector.tensor_scalar_mul(inv[:], inv[:], 0.5)
                        if j == 0:
                            nc.vector.tensor_scalar_mul(attn[:], esb[:], inv[:])
                        else:
                            nc.vector.scalar_tensor_tensor(
                                attn[:], esb[:], inv[:], attn[:],
                                op0=ALU.mult, op1=ALU.add)

                # vT (value, transposed, with ones column via pv_aug)
                for i in range(L):
                    for half in range(2):
                        pt = ps_small.tile([128, 264], F32, tag="psmall")
                        for mq in range(4):
                            mc = half * 4 + mq
                            nc.tensor.matmul(
                                pt[:, mq * 66:(mq + 1) * 66],
                                Taug[i][:, mc * 128:(mc + 1) * 128],
                                paug["pv", i][:],
                            )
                        nc.vector.tensor_copy(
                            vT_sb[i][:, half * 4:(half + 1) * 4, :], pt[:])

                # CAM attention @ value
                for i in range(L):
                    attnb = work.tile([64, 64], BF, tag="cam_attnb")
                    nc.vector.tensor_copy(attnb[:], attns[i][:])
                    ptr = ps_small.tile([64, 64], BF, tag="psmall")
                    nc.tensor.transpose(ptr[:], attnb[:], idb[:])
                    attnT = work.tile([64, 64], BF, tag="cam_attnT")
                    nc.vector.tensor_copy(attnT[:], ptr[:])
                    po = ps_big.tile([64, N], F32, tag="pbig")
                    for h in (0, 512):
                        nc.tensor.matmul(po[:, h:h + 512], attnT[:],
                                         vc_sb[i][:, h:h + 512])
                    nc.vector.tensor_copy(camv[i][:], po[:])

            # ================= Phase 2: PAM =================
            with tc.tile_pool(name="ps_e", bufs=2, space="PSUM") as ps_e, \
                 tc.tile_pool(name="ps_o", bufs=2, space="PSUM") as ps_o:
                for i in range(L):
                    for j in range(L):
                        expT = bigp.tile([128, 8 * 1024], BF, tag="big")
                        expT3 = expT[:].rearrange("p (m n) -> p m n", m=8)
                        po = ps_o.tile([65, N], F32, tag="po")
                        for mc in range(8):
                            pe = ps_e.tile([128, N], F32, tag="pe")
                            # two concurrent row-group matmuls (K=64 each):
                            # rows 0-63 compute columns 0-512, rows 64-127
                            # (replicated copies) compute columns 512-1024.
                            nc.tensor.matmul(
                                pe[:, 0:512],
                                k_sb[j][0:64, mc * 128:(mc + 1) * 128],
                                q_sb[i][0:64, 0:512],
                            )
                            nc.tensor.matmul(
                                pe[:, 512:1024],
                                k_sb[j][64:128, mc * 128:(mc + 1) * 128],
                                q_sb[i][64:128, 512:1024],
                            )
                            nc.scalar.activation(expT3[:, mc, :], pe[:], AF.Exp)
                            for h in (0, 512):
                                nc.tensor.matmul(
                                    po[:, h:h + 512],
                                    vT_sb[i][:, mc, 0:65],
                                    expT3[:, mc, h:h + 512],
                                    start=(mc == 0), stop=(mc == 7),
                                )
                        # rowsum -> per-lane layout, cheap wide reciprocal
                        rs1 = work.tile([1, N], F32, tag="pam_rs1")
                        nc.vector.tensor_copy(rs1[:], po[64:65, :])
                        rs8 = work.tile([128, 8], F32, tag="pam_rs8")
                        nc.sync.dma_start(rs8[:], rs1[:])
                        nc.vector.reciprocal(rs8[:], rs8[:])
                        nc.vector.tensor_scalar_mul(rs8[:], rs8[:], 0.5)
                        inv = work.tile([1, N], F32, tag="pam_inv")
                        nc.sync.dma_start(inv[:], rs8[:])
                        invb = work.tile([64, N], F32, tag="pam_invb")
                        nc.gpsimd.partition_broadcast(invb[:], inv[:])
                        if j == 0:
                            nc.vector.tensor_mul(acc[i][:], po[0:64, :],
                                                 invb[:])
                        else:
                            tmp = work.tile([64, N], F32, tag="pam_tmp")
                            nc.vector.tensor_mul(tmp[:], po[0:64, :], invb[:])
                            nc.vector.tensor_add(acc[i][:], acc[i][:], tmp[:])
                    fused = work.tile([64, N], F32, tag="fused")
                    nc.vector.tensor_add(fused[:], acc[i][:], camv[i][:])
                    nc.sync.dma_start(tgts_r[2 * i], fused[:])

    nc.compile()
    return nc


def _pack_shared(inputs):
    f32 = np.float32
    shared = {}
    # wall: [128, 30, 64] = w0s(9) w0t(9) w1s(2) w1t(2) w2s(4) w2t(4),
    # each tap/chunk as [ic(128), oc(64)]
    wall = np.zeros((128, 30, 64), f32)
    wall[:, 0:9] = np.transpose(
        np.asarray(inputs["fs0_w"], f32), (2, 3, 1, 0)).reshape(9, 128, 64) \
        .transpose(1, 0, 2)
    wall[:, 9:18] = np.transpose(
        np.asarray(inputs["ft0_w"], f32), (2, 3, 1, 0)).reshape(9, 128, 64) \
        .transpose(1, 0, 2)
    off = 18
    for key, nk in (("fs1_w", 2), ("ft1_w", 2), ("fs2_w", 4), ("ft2_w", 4)):
        w = np.asarray(inputs[key], f32)[:, :, 0, 0]  # [64, ic]
        wall[:, off:off + nk] = np.ascontiguousarray(w.T).reshape(
            nk, 128, 64).transpose(1, 0, 2)
        off += nk
    shared["wall"] = wall
    # pall: [65, 774] = pq(3x64) pk(3x64) cv(3x64) pv(3x66)
    pall = np.zeros((65, 774), f32)
    col = 0
    for wkey, bkey in (("pq_w", "pq_b"), ("pk_w", "pk_b"), ("cv_w", "cv_b")):
        for i in range(L):
            pall[:64, col:col + 64] = np.asarray(inputs[wkey], f32)[i].T
            pall[64, col:col + 64] = np.asarray(inputs[bkey], f32)[i]
            col += 64
    for i in range(L):
        pall[:64, col:col + 64] = np.asarray(inputs["pv_w"], f32)[i].T
        pall[64, col:col + 64] = np.asarray(inputs["pv_b"], f32)[i]
        pall[64, col + 64] = 1.0
        col += 66
    shared["pall"] = pall
    cnst = np.zeros((64, 70), f32)
    cnst[:, 0:6] = np.stack(
        [np.asarray(inputs[k], f32) for k in
         ("fs0_b", "ft0_b", "fs1_b", "ft1_b", "fs2_b", "ft2_b")], axis=1)
    cnst[:, 6:70] = np.eye(64, dtype=f32)
    shared["cnst"] = cnst
    shared["onesr"] = np.ones((1, 1024), f32)
    shared["identb"] = np.eye(64, dtype=f32).astype(ml_dtypes.bfloat16)
    return shared


def _pad0(x):
    out = np.zeros((x.shape[0], 66, 66), np.float32)
    out[:, 1:65, 1:65] = x
    return out


def kernel(**inputs):
    if "nc" not in _CACHED:
        _CACHED["nc"] = build_bass()
    nc = _CACHED["nc"]

    shared = _pack_shared(inputs)
    in_maps = []
    for b in range(B):
        m = dict(shared)
        m["s0p"] = _pad0(np.asarray(inputs["src0"][b], np.float32))
        m["t0p"] = _pad0(np.asarray(inputs["tgt0"][b], np.float32))
        m["s1"] = np.ascontiguousarray(
            np.asarray(inputs["src1"][b], np.float32).reshape(2, 128, 1024))
        m["t1"] = np.ascontiguousarray(
            np.asarray(inputs["tgt1"][b], np.float32).reshape(2, 128, 1024))
        m["s2"] = np.ascontiguousarray(
            np.asarray(inputs["src2"][b], np.float32).reshape(4, 128, 1024))
        m["t2"] = np.ascontiguousarray(
            np.asarray(inputs["tgt2"][b], np.float32).reshape(4, 128, 1024))
        in_maps.append(m)

    res = run_bass_kernel_spmd(nc, in_maps, list(range(B)))
    srcs = np.stack([np.asarray(res.results[b]["srcs"], np.float32)
                     for b in range(B)], axis=1)
    tgts = np.stack([np.asarray(res.results[b]["tgts"], np.float32)
                     for b in range(B)], axis=1)

    # ---- alpha on host from exact S/T ----
    S = srcs.reshape(L, B, C, N).astype(np.float64)
    T = tgts[1::2].reshape(L, B, C, N).astype(np.float64)
    Tbar = T.mean(axis=3)  # [L,B,C]
    Sbar = S.mean(axis=3)
    pq_w = np.asarray(inputs["pq_w"], np.float64)
    pq_b = np.asarray(inputs["pq_b"], np.float64)
    pk_w = np.asarray(inputs["pk_w"], np.float64)
    pk_b = np.asarray(inputs["pk_b"], np.float64)
    qbar = np.einsum("ioc,ibc->ibo", pq_w, Tbar) + pq_b[:, None, :]
    kbar = np.einsum("joc,jbc->jbo", pk_w, Sbar) + pk_b[:, None, :]
    e_pam_mean = np.einsum("ibo,jbo->ij", qbar, kbar) / B
    Tc = T.mean(axis=2)  # [L,B,N]
    Sc = S.mean(axis=2)
    e_cam_mean = np.einsum("ibn,jbn->ij", Tc, Sc) / B

    def _softmax(x):
        x = x - x.max(axis=-1, keepdims=True)
        e = np.exp(x)
        return e / e.sum(axis=-1, keepdims=True)

    alpha = 0.5 * (_softmax(e_cam_mean) + _softmax(e_pam_mean))
    alpha = alpha.astype(np.float32)

    return (srcs, alpha, tgts)


# revision 18
# speedup vs baseline: 1.3710x; 1.3710x over previous
"""Trainium2 Bass kernel for DAAttention (PAM + CAM dual attention, L=3 layer
pairs, B=8, C=64, N=1024), batch-parallel across 8 NeuronCores.

Per core (one batch element):
  - 6 projection convs (stride-2 3x3 for layer 0 as 9 tap matmuls over a
    host-padded input; 1x1 convs as chunked matmuls) -> S[i], T[i] [64,1024]
  - PAM: eT[i,j] = k[j]^T q[i] computed transposed (m on partitions) so both
    the softmax normalizer and the attention-value product are matmuls.
    exp without max subtraction (logits ~N(0,8), fp32-safe); row sums ride
    the value matmul through an appended ones row; normalization applied as
    a per-j rescale of the value-product output. The two 512-column energy
    matmuls run concurrently in distinct PE row groups (K=64 each) using
    partition-replicated q/k copies.
  - CAM: e_cam[i,j] = T[i] S[j]^T via PE-transposed copies; standard
    free-axis softmax; attention applied via one more matmul.
  - fp16 compute throughout (fp32 PSUM accumulate; exp in fp32; exp values
    stored bf16 for range), fp16 outputs upcast on the host.
  - alpha ([3,3] softmaxed means) is reconstructed on the host from the
    S/T outputs (means commute with the bilinear forms).
"""

import os
import sys

import numpy as np

for _p in ("/opt/trn_rl_repo", "/root/.axon_site/_ro/trn_rl_repo"):
    if os.path.isdir(_p) and _p not in sys.path:
        sys.path.insert(0, _p)

import concourse.bacc as bacc
import concourse.bass as bass  # noqa: F401
import concourse.mybir as mybir
from concourse import tile
from concourse.bass_utils import run_bass_kernel_spmd

L, C, H, W = 3, 64, 32, 32
N = H * W  # 1024
B = 8
F32 = mybir.dt.float32
F16 = mybir.dt.float16
BF = mybir.dt.bfloat16
AF = mybir.ActivationFunctionType
ALU = mybir.AluOpType
AX = mybir.AxisListType

_CACHED = {}


def build_bass():
    nc = bacc.Bacc("TRN2", target_bir_lowering=False, debug=False, num_devices=B)

    # ---- DRAM parameters (per-core inputs, fp16) ----
    s0p = nc.dram_tensor("s0p", [128, 66, 66], F16, kind="ExternalInput")
    t0p = nc.dram_tensor("t0p", [128, 66, 66], F16, kind="ExternalInput")
    s1 = nc.dram_tensor("s1", [2, 128, 1024], F16, kind="ExternalInput")
    t1 = nc.dram_tensor("t1", [2, 128, 1024], F16, kind="ExternalInput")
    s2 = nc.dram_tensor("s2", [4, 128, 1024], F16, kind="ExternalInput")
    t2 = nc.dram_tensor("t2", [4, 128, 1024], F16, kind="ExternalInput")
    # wall: [128, 30, 64] = w0s(9) w0t(9) w1s(2) w1t(2) w2s(4) w2t(4)
    wall = nc.dram_tensor("wall", [128, 30, 64], F16, kind="ExternalInput")
    # pall: [65, 838] = pq(3x64) pk(3x64) cv(3x64) pv(3x66) ident(64)
    pall = nc.dram_tensor("pall", [65, 838], F16, kind="ExternalInput")
    cnst = nc.dram_tensor("cnst", [64, 6], F32, kind="ExternalInput")
    ones16 = nc.dram_tensor("ones16", [1, 1024], F16, kind="ExternalInput")

    srcs = nc.dram_tensor("srcs", [L, C * N], F16, kind="ExternalOutput")
    tgts = nc.dram_tensor("tgts", [2 * L, C * N], F16, kind="ExternalOutput")
    srcs_r = srcs.ap().rearrange("l (c n) -> l c n", c=C)
    tgts_r = tgts.ap().rearrange("l (c n) -> l c n", c=C)

    with tile.TileContext(nc) as tc:
        with (
            tc.tile_pool(name="persist", bufs=1) as pers,
            tc.tile_pool(name="wpool", bufs=1) as wpool,
            tc.tile_pool(name="big", bufs=4) as bigp,
            tc.tile_pool(name="work", bufs=2) as work,
        ):
            # ---- persistent SBUF tensors ----
            Saug = [pers.tile([65, N], F16, tag=f"Saug{i}", name=f"Saug{i}")
                    for i in range(L)]
            Taug = [pers.tile([65, N], F16, tag=f"Taug{i}", name=f"Taug{i}")
                    for i in range(L)]
            q_sb = [pers.tile([128, N], F16, tag=f"q{i}", name=f"q{i}")
                    for i in range(L)]
            k_sb = [pers.tile([128, N], F16, tag=f"k{i}", name=f"k{i}")
                    for i in range(L)]
            vc_sb = [pers.tile([64, N], F16, tag=f"vc{i}", name=f"vc{i}")
                    for i in range(L)]
            vT_sb = [pers.tile([128, 8, 66], BF, tag=f"vT{i}", name=f"vT{i}")
                     for i in range(L)]
            TTb = [pers.tile([128, 8, 64], F16, tag=f"TT{i}", name=f"TT{i}")
                   for i in range(L)]
            STb = [pers.tile([128, 8, 64], F16, tag=f"ST{i}", name=f"ST{i}")
                   for i in range(L)]
            camv = [pers.tile([64, N], F32, tag=f"camv{i}", name=f"camv{i}")
                    for i in range(L)]
            acc = [pers.tile([64, N], F32, tag=f"acc{i}", name=f"acc{i}")
                   for i in range(L)]

            # ---- input loads first (conv inputs are the critical path) ----
            big_xin = {}
            for nm, hndl, sz in (("s0", s0p, 66 * 66), ("t0", t0p, 66 * 66),
                                 ("s1", s1, 2048), ("t1", t1, 2048),
                                 ("s2", s2, 4096), ("t2", t2, 4096)):
                xin = bigp.tile([128, sz], F16, tag="big", name=f"xin_{nm}")
                big_xin[nm] = xin
                if sz == 66 * 66:
                    nc.sync.dma_start(
                        xin[:], hndl.ap().rearrange("p a b -> p (a b)"))
                else:
                    nk = sz // 1024
                    nc.sync.dma_start(
                        xin[:].rearrange("p (k n) -> p k n", k=nk),
                        hndl.ap().transpose([1, 0, 2]))

            wall_sb = wpool.tile([128, 30, 64], F16, tag="wall")
            nc.sync.dma_start(wall_sb[:], wall.ap())
            pall_sb = wpool.tile([65, 838], F16, tag="pall")
            nc.sync.dma_start(pall_sb[:], pall.ap())
            cnst_sb = wpool.tile([64, 6], F32, tag="cnst")
            nc.sync.dma_start(cnst_sb[:], cnst.ap())

            cb = cnst_sb[:]
            id16 = pall_sb[0:64, 774:838]
            paug = {}
            for gi, nm in enumerate(("pq", "pk", "cv")):
                for i in range(L):
                    paug[nm, i] = pall_sb[:, (gi * 3 + i) * 64:
                                          (gi * 3 + i + 1) * 64]
            for i in range(L):
                paug["pv", i] = pall_sb[:, 576 + i * 66:576 + (i + 1) * 66]
            woff = {"w0s": 0, "w0t": 9, "w1s": 18, "w1t": 20,
                    "w2s": 22, "w2t": 26}
            wsb = {nm: wall_sb[:, o:o + {"w0s": 9, "w0t": 9, "w1s": 2,
                                         "w1t": 2, "w2s": 4, "w2t": 4}[nm], :]
                   for nm, o in woff.items()}

            with tc.tile_pool(name="ps_big", bufs=2, space="PSUM") as ps_big, \
                 tc.tile_pool(name="ps_small", bufs=4, space="PSUM") as ps_small:

                # ================= Phase 1: convs =================
                def conv0(dst, xin, wname, bcol):
                    xv = xin.rearrange("p (a b) -> p a b", a=66)
                    pt = ps_big.tile([64, N], F32, tag="pbig")
                    pt3 = pt[:].rearrange("c (h w) -> c h w", h=32)
                    for ti, (kh, kw) in enumerate(
                            (a, b) for a in range(3) for b in range(3)):
                        for h0 in (0, 16):
                            nc.tensor.matmul(
                                pt3[:, h0:h0 + 16, :],
                                wsb[wname][:, kh * 3 + kw, :],
                                xv[:, kh + 2 * h0:kh + 2 * h0 + 32:2,
                                   kw:kw + 64:2],
                                start=(ti == 0), stop=(ti == 8),
                            )
                    nc.scalar.activation(dst[0:64, :], pt[:], AF.Identity,
                                         bias=cb[:, bcol:bcol + 1])
                    nc.sync.dma_start(dst[64:65, :], ones16.ap())

                def conv1x1(dst, xin, wname, nk, bcol):
                    xv = xin.rearrange("p (k n) -> p k n", k=nk)
                    pt = ps_big.tile([64, N], F32, tag="pbig")
                    for kc in range(nk):
                        for h in (0, 512):
                            nc.tensor.matmul(
                                pt[:, h:h + 512],
                                wsb[wname][:, kc, :],
                                xv[:, kc, h:h + 512],
                                start=(kc == 0), stop=(kc == nk - 1),
                            )
                    nc.scalar.activation(dst[0:64, :], pt[:], AF.Identity,
                                         bias=cb[:, bcol:bcol + 1])
                    nc.sync.dma_start(dst[64:65, :], ones16.ap())

                conv0(Saug[0], big_xin["s0"][:], "w0s", 0)
                conv0(Taug[0], big_xin["t0"][:], "w0t", 1)
                conv1x1(Saug[1], big_xin["s1"][:], "w1s", 2, 2)
                conv1x1(Taug[1], big_xin["t1"][:], "w1t", 2, 3)
                conv1x1(Saug[2], big_xin["s2"][:], "w2s", 4, 4)
                conv1x1(Taug[2], big_xin["t2"][:], "w2t", 4, 5)

                # write out S (srcs) and T (odd rows of tgts)
                for i in range(L):
                    nc.sync.dma_start(srcs_r[i], Saug[i][0:64, :])
                    nc.sync.dma_start(tgts_r[2 * i + 1], Taug[i][0:64, :])

                # ================= Phase 1b: projections =================
                def proj(dst, aug, xaug, replicate=False):
                    pt = ps_big.tile([64, N], F32, tag="pbig")
                    for h in (0, 512):
                        nc.tensor.matmul(pt[:, h:h + 512], aug,
                                         xaug[:, h:h + 512])
                    nc.vector.tensor_copy(dst[0:64, :], pt[:])
                    if replicate:
                        nc.sync.dma_start(dst[64:128, :], dst[0:64, :])

                for i in range(L):
                    proj(q_sb[i][:], paug["pq", i], Taug[i][:], replicate=True)
                    proj(k_sb[i][:], paug["pk", i], Saug[i][:], replicate=True)
                    proj(vc_sb[i][:], paug["cv", i], Taug[i][:])

                # transposed S/T (fp16) for CAM
                for i in range(L):
                    for (dst, srcT) in ((TTb[i], Taug[i]), (STb[i], Saug[i])):
                        pt = ps_small.tile([128, 512], F16, tag="psmall")
                        for mc in range(8):
                            nc.tensor.transpose(
                                pt[:, mc * 64:(mc + 1) * 64],
                                srcT[0:64, mc * 128:(mc + 1) * 128],
                                id16,
                            )
                        nc.vector.tensor_copy(dst[:], pt[:])

                # ================= Phase 1c: CAM energies + softmax =========
                attns = []
                for i in range(L):
                    attn = work.tile([64, 64], F32, tag=f"cam_attn{i}",
                                     name=f"cam_attn{i}", bufs=1)
                    attns.append(attn)
                    for j in range(L):
                        pe = ps_small.tile([64, 64], F32, tag="psmall")
                        for nt in range(8):
                            nc.tensor.matmul(pe[:], TTb[i][:, nt, :],
                                             STb[j][:, nt, :],
                                             start=(nt == 0), stop=(nt == 7))
                        negmax = work.tile([64, 1], F32, tag="cam_negmax")
                        nc.vector.tensor_reduce(negmax[:], pe[:], axis=AX.X,
                                                op=ALU.max, negate=True)
                        esb = work.tile([64, 64], F32, tag="cam_exp")
                        ssum = work.tile([64, 1], F32, tag="cam_sum")
                        nc.scalar.activation(esb[:], pe[:], AF.Exp,
                                             bias=negmax[:], accum_out=ssum[:])
                        inv = work.tile([64, 1], F32, tag="cam_inv")
                        nc.vector.reciprocal(inv[:], ssum[:])
                        nc.vector.tensor_scalar_mul(inv[:], inv[:], 0.5)
                        if j == 0:
                            nc.vector.tensor_scalar_mul(attn[:], esb[:], inv[:])
                        else:
                            nc.vector.scalar_tensor_tensor(
                                attn[:], esb[:], inv[:], attn[:],
                                op0=ALU.mult, op1=ALU.add)

                # vT (value, transposed, with ones column via pv_aug)
                for i in range(L):
                    for half in range(2):
                        pt = ps_small.tile([128, 264], F32, tag="psmall")
                        for mq in range(4):
                            mc = half * 4 + mq
                            nc.tensor.matmul(
                                pt[:, mq * 66:(mq + 1) * 66],
                                Taug[i][:, mc * 128:(mc + 1) * 128],
                                paug["pv", i],
                            )
                        nc.vector.tensor_copy(
                            vT_sb[i][:, half * 4:(half + 1) * 4, :], pt[:])

                # CAM attention @ value
                for i in range(L):
                    attnb = work.tile([64, 64], F16, tag="cam_attnb")
                    nc.vector.tensor_copy(attnb[:], attns[i][:])
                    ptr = ps_small.tile([64, 64], F16, tag="psmall")
                    nc.tensor.transpose(ptr[:], attnb[:], id16)
                    attnT = work.tile([64, 64], F16, tag="cam_attnT")
                    nc.vector.tensor_copy(attnT[:], ptr[:])
                    po = ps_big.tile([64, N], F32, tag="pbig")
                    for h in (0, 512):
                        nc.tensor.matmul(po[:, h:h + 512], attnT[:],
                                         vc_sb[i][:, h:h + 512])
                    nc.vector.tensor_copy(camv[i][:], po[:])

            # ================= Phase 2: PAM =================
            with tc.tile_pool(name="ps_e", bufs=2, space="PSUM") as ps_e, \
                 tc.tile_pool(name="ps_o", bufs=2, space="PSUM") as ps_o:
                for i in range(L):
                    for j in range(L):
                        expT = bigp.tile([128, 8 * 1024], BF, tag="big")
                        expT3 = expT[:].rearrange("p (m n) -> p m n", m=8)
                        po = ps_o.tile([65, N], F32, tag="po")
                        for mc in range(8):
                            pe = ps_e.tile([128, N], F32, tag="pe")
                            # two concurrent row-group matmuls (K=64 each)
                            nc.tensor.matmul(
                                pe[:, 0:512],
                                k_sb[j][0:64, mc * 128:(mc + 1) * 128],
                                q_sb[i][0:64, 0:512],
                            )
                            nc.tensor.matmul(
                                pe[:, 512:1024],
                                k_sb[j][64:128, mc * 128:(mc + 1) * 128],
                                q_sb[i][64:128, 512:1024],
                            )
                            nc.scalar.activation(expT3[:, mc, :], pe[:], AF.Exp)
                            for h in (0, 512):
                                nc.tensor.matmul(
                                    po[:, h:h + 512],
                                    vT_sb[i][:, mc, 0:65],
                                    expT3[:, mc, h:h + 512],
                                    start=(mc == 0), stop=(mc == 7),
                                )
                        # rowsum -> per-lane layout, cheap wide reciprocal
                        rs1 = work.tile([1, N], F32, tag="pam_rs1")
                        nc.vector.tensor_copy(rs1[:], po[64:65, :])
                        rs8 = work.tile([128, 8], F32, tag="pam_rs8")
                        nc.sync.dma_start(rs8[:], rs1[:])
                        nc.vector.reciprocal(rs8[:], rs8[:])
                        nc.vector.tensor_scalar_mul(rs8[:], rs8[:], 0.5)
                        inv = work.tile([1, N], F32, tag="pam_inv")
                        nc.sync.dma_start(inv[:], rs8[:])
                        invb = work.tile([64, N], F32, tag="pam_invb")
                        nc.gpsimd.partition_broadcast(invb[:], inv[:])
                        if j == 0:
                            nc.vector.tensor_mul(acc[i][:], po[0:64, :],
                                                 invb[:])
                        else:
                            tmp = work.tile([64, N], F32, tag="pam_tmp")
                            nc.vector.tensor_mul(tmp[:], po[0:64, :], invb[:])
                            nc.vector.tensor_add(acc[i][:], acc[i][:], tmp[:])
                    fused = work.tile([64, N], F16, tag="fused")
                    nc.vector.tensor_add(fused[:], acc[i][:], camv[i][:])
                    nc.sync.dma_start(tgts_r[2 * i], fused[:])

    nc.compile()
    return nc


def _pack_shared(inputs):
    f16 = np.float16
    f32 = np.float32
    shared = {}
    # wall: [128, 30, 64], each tap/chunk as [ic(128), oc(64)]
    wall = np.zeros((128, 30, 64), f32)
    wall[:, 0:9] = np.transpose(
        np.asarray(inputs["fs0_w"], f32), (2, 3, 1, 0)).reshape(9, 128, 64) \
        .transpose(1, 0, 2)
    wall[:, 9:18] = np.transpose(
        np.asarray(inputs["ft0_w"], f32), (2, 3, 1, 0)).reshape(9, 128, 64) \
        .transpose(1, 0, 2)
    off = 18
    for key, nk in (("fs1_w", 2), ("ft1_w", 2), ("fs2_w", 4), ("ft2_w", 4)):
        w = np.asarray(inputs[key], f32)[:, :, 0, 0]  # [64, ic]
        wall[:, off:off + nk] = np.ascontiguousarray(w.T).reshape(
            nk, 128, 64).transpose(1, 0, 2)
        off += nk
    shared["wall"] = wall.astype(f16)
    # pall: [65, 838] = pq(3x64) pk(3x64) cv(3x64) pv(3x66) ident(64)
    pall = np.zeros((65, 838), f32)
    col = 0
    for wkey, bkey in (("pq_w", "pq_b"), ("pk_w", "pk_b"), ("cv_w", "cv_b")):
        for i in range(L):
            pall[:64, col:col + 64] = np.asarray(inputs[wkey], f32)[i].T
            pall[64, col:col + 64] = np.asarray(inputs[bkey], f32)[i]
            col += 64
    for i in range(L):
        pall[:64, col:col + 64] = np.asarray(inputs["pv_w"], f32)[i].T
        pall[64, col:col + 64] = np.asarray(inputs["pv_b"], f32)[i]
        pall[64, col + 64] = 1.0
        col += 66
    pall[:64, 774:838] = np.eye(64, dtype=f32)
    shared["pall"] = pall.astype(f16)
    shared["cnst"] = np.ascontiguousarray(np.stack(
        [np.asarray(inputs[k], f32) for k in
         ("fs0_b", "ft0_b", "fs1_b", "ft1_b", "fs2_b", "ft2_b")], axis=1))
    shared["ones16"] = np.ones((1, 1024), f16)
    return shared


def _pad0(x):
    out = np.zeros((x.shape[0], 66, 66), np.float16)
    out[:, 1:65, 1:65] = x
    return out


def kernel(**inputs):
    if "nc" not in _CACHED:
        _CACHED["nc"] = build_bass()
    nc = _CACHED["nc"]

    shared = _pack_shared(inputs)
    in_maps = []
    for b in range(B):
        m = dict(shared)
        m["s0p"] = _pad0(np.asarray(inputs["src0"][b], np.float32))
        m["t0p"] = _pad0(np.asarray(inputs["tgt0"][b], np.float32))
        m["s1"] = np.asarray(inputs["src1"][b], np.float16).reshape(
            2, 128, 1024).copy()
        m["t1"] = np.asarray(inputs["tgt1"][b], np.float16).reshape(
            2, 128, 1024).copy()
        m["s2"] = np.asarray(inputs["src2"][b], np.float16).reshape(
            4, 128, 1024).copy()
        m["t2"] = np.asarray(inputs["tgt2"][b], np.float16).reshape(
            4, 128, 1024).copy()
        in_maps.append(m)

    res = run_bass_kernel_spmd(nc, in_maps, list(range(B)))
    srcs = np.stack([np.asarray(res.results[b]["srcs"], np.float32)
                     for b in range(B)], axis=1)
    tgts = np.stack([np.asarray(res.results[b]["tgts"], np.float32)
                     for b in range(B)], axis=1)

    # ---- alpha on host from S/T ----
    S = srcs.reshape(L, B, C, N).astype(np.float64)
    T = tgts[1::2].reshape(L, B, C, N).astype(np.float64)
    Tbar = T.mean(axis=3)  # [L,B,C]
    Sbar = S.mean(axis=3)
    pq_w = np.asarray(inputs["pq_w"], np.float64)
    pq_b = np.asarray(inputs["pq_b"], np.float64)
    pk_w = np.asarray(inputs["pk_w"], np.float64)
    pk_b = np.asarray(inputs["pk_b"], np.float64)
    qbar = np.einsum("ioc,ibc->ibo", pq_w, Tbar) + pq_b[:, None, :]
    kbar = np.einsum("joc,jbc->jbo", pk_w, Sbar) + pk_b[:, None, :]
    e_pam_mean = np.einsum("ibo,jbo->ij", qbar, kbar) / B
    Tc = T.mean(axis=2)  # [L,B,N]
    Sc = S.mean(axis=2)
    e_cam_mean = np.einsum("ibn,jbn->ij", Tc, Sc) / B

    def _softmax(x):
        x = x - x.max(axis=-1, keepdims=True)
        e = np.exp(x)
        return e / e.sum(axis=-1, keepdims=True)

    alpha = 0.5 * (_softmax(e_cam_mean) + _softmax(e_pam_mean))
    alpha = alpha.astype(np.float32)

    return (srcs, alpha, tgts)
